# revision 6
# baseline (speedup 1.0000x reference)
"""GCN (3-layer, GCN-norm, jumping-knowledge cat, global-add-pool, MLP head)
on 8 Trainium2 NeuronCores via Bass/Tile.

Strategy:
  - Nodes assigned round-robin to 8 cores (node n -> core n%8).  Within a
    core, nodes are ordered by "low-half in-degree" (d0) descending; this
    "table order" is used for everything on-device.
  - Per conv layer: each core computes G~ = dinv * (H @ W) for its node
    shard (bf16, rows padded to 128 cols), AllGather -> full message table
    [V, 128] bf16 in each core's DRAM.
  - Message aggregation: per 128-node block, a fixed number K(b) of gather
    slots (ELL, degree-sorted so padding is small).  dma_gather pulls
    128*K(b) rows (token t -> partition t%128, slot t//128), a strided DVE
    reduce sums the K slots -> per-node partial sum.
  - dma_gather indices are int16, so the table is addressed in two halves
    (cores 0-3 / cores 4-7).  The low half is aggregated in table order
    (kept in SBUF); the high half is aggregated in a separate d1-sorted
    order, written to DRAM, and realigned with one more K=1 gather.
  - Post: out = relu(dinv * S + b) applied feature-major (after a PE
    transpose) so the per-feature bias is a per-partition scalar.
  - Pool: per-core partial graph sums via the same gather trick over the
    JK output table, AllReduce [512,128], then the tiny MLP redundantly.
"""
import sys
import os

sys.path.insert(0, "/opt/trn_rl_repo")
os.environ.setdefault("JAX_PLATFORMS", "")

import numpy as np
import ml_dtypes

import concourse.bass as bass
import concourse.bacc as bacc
import concourse.mybir as mybir
import concourse.tile as tile
from concourse._compat import get_trn_type
from concourse.bass_utils import run_bass_kernel_spmd
from concourse.masks import make_identity

N_NODES = 50000
N_EDGES = 800000
N_GRAPHS = 500
IN_DIM = 128
HID = 96
OUT_DIM = 64
NC = 8

JOB_SLOTS = 32  # max gather slots (of 128 tokens) per dma_gather call

BF16 = ml_dtypes.bfloat16


def _pack16(flat):
    """[T] int16 (T%128==0) -> [128, T//16] wrapped-in-16, replicated x8."""
    t = np.ascontiguousarray(flat.reshape(-1, 16).T)
    return np.ascontiguousarray(np.tile(t, (8, 1)))


class Plan:
    pass


def _plan(edge_index, batch, n_nodes=N_NODES, n_graphs=N_GRAPHS):
    """All host-side index preprocessing (pure numpy)."""
    p = Plan()
    N = n_nodes
    assert N % NC == 0
    NLOC = N // NC
    NB = (NLOC + 1 + 127) // 128  # >=1 pad row per core
    NT = NB * 128
    V = NC * NT
    HALF = (NC // 2) * NT
    assert HALF < 32768, f"half table {HALF} exceeds int16 range"
    ZROW = NLOC  # per-half-local index of a guaranteed-zero row
    GB = (n_graphs + 127) // 128
    NG = GB * 128

    src = np.asarray(edge_index[0], np.int64)
    dst = np.asarray(edge_index[1], np.int64)
    batch = np.asarray(batch, np.int64)

    deg = np.bincount(dst, minlength=N).astype(np.float64) + 1.0
    dinv_node = (1.0 / np.sqrt(deg)).astype(np.float32)

    loop = np.arange(N, dtype=np.int64)
    sa = np.concatenate([src, loop])
    da = np.concatenate([dst, loop])
    core_d = (da % NC).astype(np.int64)
    loc_d = da // NC
    half_s = ((sa % NC) >= (NC // 2))  # True -> high half

    d0 = np.zeros((NC, NLOC), np.int64)
    d1 = np.zeros((NC, NLOC), np.int64)
    for c in range(NC):
        m = core_d == c
        dl = loc_d[m]
        hs = half_s[m]
        d0[c] = np.bincount(dl[~hs], minlength=NLOC)
        d1[c] = np.bincount(dl[hs], minlength=NLOC)

    rank0_order = np.argsort(-d0, axis=1, kind="stable")  # [NC, NLOC]
    rank1_order = np.argsort(-d1, axis=1, kind="stable")
    rank0_pos = np.empty_like(rank0_order)
    rank1_pos = np.empty_like(rank1_order)
    ar = np.arange(NLOC)
    for c in range(NC):
        rank0_pos[c, rank0_order[c]] = ar
        rank1_pos[c, rank1_order[c]] = ar

    ns = np.arange(N)
    trow = (ns % NC) * NT + rank0_pos[ns % NC, ns // NC]  # global table row
    tok_of_node = (trow - (trow >= HALF) * HALF).astype(np.int64)

    d0s = -np.sort(-d0, axis=1)
    d1s = -np.sort(-d1, axis=1)
    K0 = np.zeros(NB, np.int64)
    K1 = np.zeros(NB, np.int64)
    for b in range(NB):
        q = 128 * b
        if q < NLOC:
            K0[b] = d0s[:, q].max()
            K1[b] = d1s[:, q].max()

    def ell_fill(rank_pos, want_high):
        Km = max(int(K0.max() if not want_high else K1.max()), 1)
        M = np.full((NC, NT, Km), ZROW, np.int16)
        for c in range(NC):
            m = (core_d == c) & (half_s == want_high)
            r = rank_pos[c, loc_d[m]]
            t = tok_of_node[sa[m]]
            o = np.argsort(r, kind="stable")
            rs = r[o]
            ts = t[o]
            j = np.arange(len(rs)) - np.searchsorted(rs, rs, "left")
            M[c, rs, j] = ts.astype(np.int16)
        return M

    M0 = ell_fill(rank0_pos, False)
    M1 = ell_fill(rank1_pos, True)

    def make_jobs(K):
        jobs = []
        cur_blocks = []
        s = 0
        for b in range(NB):
            k = int(K[b])
            if cur_blocks and s + k > JOB_SLOTS:
                jobs.append(cur_blocks)
                cur_blocks = []
                s = 0
            cur_blocks.append(b)
            s += k
        if cur_blocks:
            jobs.append(cur_blocks)
        # drop jobs whose total K is 0
        return [bl for bl in jobs if sum(int(K[b]) for b in bl) > 0]

    jobs0 = make_jobs(K0)
    jobs1 = make_jobs(K1)

    def tokens_for(M, jobs, K):
        per_core = []
        for c in range(NC):
            parts = []
            for bl in jobs:
                for b in bl:
                    k = int(K[b])
                    if k == 0:
                        continue
                    blk = M[c, 128 * b : 128 * (b + 1), :k]  # [128, k]
                    parts.append(np.ascontiguousarray(blk.T).reshape(-1))
            flat = np.concatenate(parts) if parts else np.zeros(0, np.int16)
            per_core.append(_pack16(flat))
        return np.stack(per_core)  # [NC, 128, T/16]

    idx_low = tokens_for(M0, jobs0, K0)
    idx_high = tokens_for(M1, jobs1, K1)

    # level-2 realign: table-order pos q -> rank1 pos of same node
    idx_l2 = np.empty((NC, NT), np.int16)
    for c in range(NC):
        l2 = np.arange(NT, dtype=np.int16)
        l2[:NLOC] = rank1_pos[c, rank0_order[c]].astype(np.int16)
        idx_l2[c] = l2
    idx_l2 = np.stack([_pack16(idx_l2[c]) for c in range(NC)])

    # pooling ELL over graphs (canonical graph order)
    cnt = np.zeros((NC, NG), np.int64)
    memb = np.full((NC, NG, 1), ZROW, np.int16)
    KP_list = []
    for c in range(NC):
        g = batch[ar * NC + c]
        cnt[c] = np.bincount(g, minlength=NG)
    KPm = max(int(cnt.max()), 1)
    memb = np.full((NC, NG, KPm), ZROW, np.int16)
    for c in range(NC):
        g = batch[ar * NC + c]
        o = np.argsort(g, kind="stable")
        gs = g[o]
        ps = rank0_pos[c, o]
        j = np.arange(len(gs)) - np.searchsorted(gs, gs, "left")
        memb[c, gs, j] = ps.astype(np.int16)
    KP = np.zeros(GB, np.int64)
    for b in range(GB):
        KP[b] = cnt[:, 128 * b : 128 * (b + 1)].max()
        KP[b] = max(int(KP[b]), 1)
    pool_jobs = [[b] for b in range(GB)]
    idx_pool = tokens_for(memb, pool_jobs, KP)

    # per-core dinv in table order, laid out [128, NB] (p, b) = pos 128b+p
    dinv_tab = np.zeros((NC, 128, NB), np.float32)
    for c in range(NC):
        dv = np.zeros(NT, np.float32)
        dv[:NLOC] = dinv_node[rank0_order[c] * NC + c]
        dinv_tab[c] = dv.reshape(NB, 128).T

    p.N, p.NLOC, p.NB, p.NT, p.V, p.HALF, p.GB, p.NG = N, NLOC, NB, NT, V, HALF, GB, NG
    p.K0, p.K1, p.KP = K0, K1, KP
    p.jobs0, p.jobs1 = jobs0, jobs1
    p.idx_low, p.idx_high, p.idx_l2, p.idx_pool = idx_low, idx_high, idx_l2, idx_pool
    p.dinv_tab = dinv_tab
    p.rank0_order = rank0_order
    p.cnt_total = np.bincount(batch, minlength=NG).astype(np.float32)
    p.n_graphs = n_graphs
    return p


def _meta(p, in_dim=IN_DIM, hid=HID, out_dim=OUT_DIM):
    """Compile-time description of the program (identical across cores)."""
    return dict(
        NB=p.NB, NT=p.NT, V=p.V, HALF=p.HALF, GB=p.GB, NG=p.NG, NLOC=p.NLOC,
        K0=[int(k) for k in p.K0], K1=[int(k) for k in p.K1],
        KP=[int(k) for k in p.KP],
        jobs0=p.jobs0, jobs1=p.jobs1,
        TL=p.idx_low.shape[2], TH=p.idx_high.shape[2], TP=p.idx_pool.shape[2],
        IN=in_dim, HID=hid, OUT=out_dim,
        n_graphs=p.n_graphs,
    )


def _build(meta):
    """Build the SPMD Bass program (one NEFF, runs on all 8 cores)."""
    NB, NT, V, HALF = meta["NB"], meta["NT"], meta["V"], meta["HALF"]
    GB, NG, NLOC = meta["GB"], meta["NG"], meta["NLOC"]
    K0, K1, KP = meta["K0"], meta["K1"], meta["KP"]
    jobs0, jobs1 = meta["jobs0"], meta["jobs1"]
    IN, HD, OUT = meta["IN"], meta["HID"], meta["OUT"]
    HP = 128  # padded feature width
    n_graphs = meta["n_graphs"]

    f32 = mybir.dt.float32
    bf16 = mybir.dt.bfloat16
    i16 = mybir.dt.int16

    nc = bacc.Bacc(
        get_trn_type() or "TRN2",
        target_bir_lowering=False,
        debug=False,
        num_devices=NC,
    )

    # ---- I/O -------------------------------------------------------------
    xT_in = nc.dram_tensor("xT", [IN, NT], bf16, kind="ExternalInput")
    w0_in = nc.dram_tensor("w0", [IN, HP], bf16, kind="ExternalInput")
    w1_in = nc.dram_tensor("w1", [HD, HP], bf16, kind="ExternalInput")
    w2_in = nc.dram_tensor("w2", [HD, HP], bf16, kind="ExternalInput")
    wjk_in = nc.dram_tensor("wjk", [3, HD, HP], bf16, kind="ExternalInput")
    wm1_in = nc.dram_tensor("wm1", [HD, HP], f32, kind="ExternalInput")
    wm2_in = nc.dram_tensor("wm2", [HD, HP], f32, kind="ExternalInput")
    bT_in = nc.dram_tensor("bT", [HD, 3], f32, kind="ExternalInput")
    bm1_in = nc.dram_tensor("bm1", [1, HP], f32, kind="ExternalInput")
    bm2_in = nc.dram_tensor("bm2", [1, HP], f32, kind="ExternalInput")
    pbias_in = nc.dram_tensor("pbias", [NG, HP], f32, kind="ExternalInput")
    dinv_in = nc.dram_tensor("dinv", [128, NB], f32, kind="ExternalInput")
    il_in = nc.dram_tensor("idx_low", [128, meta["TL"]], i16, kind="ExternalInput")
    ih_in = nc.dram_tensor("idx_high", [128, meta["TH"]], i16, kind="ExternalInput")
    i2_in = nc.dram_tensor("idx_l2", [128, NT // 16], i16, kind="ExternalInput")
    ip_in = nc.dram_tensor("idx_pool", [128, meta["TP"]], i16, kind="ExternalInput")
    out_ext = nc.dram_tensor("out", [n_graphs, OUT], f32, kind="ExternalOutput")

    with tile.TileContext(nc) as tc:
        with (
            tc.tile_pool(name="dram", bufs=1, space="DRAM") as dram,
            tc.tile_pool(name="const", bufs=1) as cst,
            tc.tile_pool(name="acts", bufs=1) as acts,
            tc.tile_pool(name="gjob", bufs=3) as gjob,
            tc.tile_pool(name="work", bufs=4) as work,
            tc.tile_pool(name="psum", bufs=6, space="PSUM") as psum,
        ):
            # ---- persistent DRAM ----------------------------------------
            tbl_shard = dram.tile([NT, HP], bf16, tag="tbl_shard")
            tbl = dram.tile([V, HP], bf16, tag="tbl")
            s_high = dram.tile([NT, HP], f32, tag="s_high")
            h_tbl = dram.tile([NT, HP], bf16, tag="h_tbl")
            ar_in = dram.tile([NG, HP], f32, tag="ar_in")
            ar_out = dram.tile([NG, HP], f32, tag="ar_out")

            # ---- constants into SBUF ------------------------------------
            def load(shape, dt, src, tag):
                t = cst.tile(shape, dt, tag=tag, name=tag)
                nc.sync.dma_start(t[:], src)
                return t

            xT = load([IN, NT], bf16, xT_in[:], "xT")
            w0 = load([IN, HP], bf16, w0_in[:], "w0")
            w1 = load([HD, HP], bf16, w1_in[:], "w1")
            w2 = load([HD, HP], bf16, w2_in[:], "w2")
            wjk = [load([HD, HP], bf16, wjk_in[l], f"wjk{l}") for l in range(3)]
            wm1 = load([HD, HP], f32, wm1_in[:], "wm1")
            wm2 = load([HD, HP], f32, wm2_in[:], "wm2")
            bT = load([HD, 3], f32, bT_in[:], "bT")
            bm1 = load([1, HP], f32, bm1_in[:], "bm1")
            bm2 = load([1, HP], f32, bm2_in[:], "bm2")
            dinv = load([128, NB], f32, dinv_in[:], "dinv")
            idx_low = load([128, meta["TL"]], i16, il_in[:], "idx_low")
            idx_high = load([128, meta["TH"]], i16, ih_in[:], "idx_high")
            idx_l2 = load([128, NT // 16], i16, i2_in[:], "idx_l2")
            idx_pool = load([128, meta["TP"]], i16, ip_in[:], "idx_pool")

            ident = cst.tile([128, 128], f32, tag="ident")
            make_identity(nc, ident[:])
            ones1 = cst.tile([1, HP], f32, tag="ones1")
            nc.vector.memset(ones1[:], 1.0)

            # persistent activations (feature-major, bf16)
            hT = [acts.tile([HD, NT], bf16, tag=f"h{l}T", name=f"h{l}T") for l in range(3)]
            s_low = acts.tile([128, NT], f32, tag="s_low")
            s2 = acts.tile([128, NB * HP], f32, tag="s2")

            layers = [(xT, IN, w0), (hT[0], HD, w1), (hT[1], HD, w2)]

            for l, (hsrc, kdim, w) in enumerate(layers):
                # -- dense: tbl_shard = dinv * (H @ W), bf16 --------------
                for b in range(NB):
                    pd = psum.tile([128, HP], f32, tag="ps")
                    nc.tensor.matmul(
                        pd[:],
                        lhsT=hsrc[:kdim, 128 * b : 128 * (b + 1)],
                        rhs=w[:kdim, :],
                        start=True,
                        stop=True,
                    )
                    tt = work.tile([128, HP], bf16, tag="tblt")
                    nc.vector.tensor_scalar_mul(tt[:], pd[:], dinv[:, b : b + 1])
                    nc.sync.dma_start(tbl_shard[128 * b : 128 * (b + 1), :], tt[:])

                nc.gpsimd.collective_compute(
                    "AllGather",
                    mybir.AluOpType.bypass,
                    ins=[tbl_shard[:].opt()],
                    outs=[tbl[:].opt()],
                    replica_groups=[list(range(NC))],
                )

                # -- high-half ELL: gather, reduce, spill to s_high -------
                col = 0
                for bl in jobs1:
                    slots = sum(K1[b] for b in bl)
                    if slots == 0:
                        continue
                    nidx = 128 * slots
                    g = gjob.tile([128, slots, HP], bf16, tag="g1")
                    nc.gpsimd.dma_gather(
                        g[:],
                        tbl[HALF:, :],
                        idx_high[:, col : col + nidx // 16],
                        nidx,
                        nidx,
                        HP,
                        single_packet=False,
                    )
                    col += nidx // 16
                    off = 0
                    for b in bl:
                        k = K1[b]
                        if k == 0:
                            continue
                        red = work.tile([128, HP], f32, tag="red1")
                        nc.vector.reduce_sum(
                            out=red[:],
                            in_=g[:, off : off + k, :].rearrange("p k d -> p d k"),
                            axis=mybir.AxisListType.X,
                        )
                        nc.sync.dma_start(
                            s_high[128 * b : 128 * (b + 1), :], red[:]
                        )
                        off += k
                covered = {b for bl in jobs1 for b in bl if K1[b] > 0}
                for b in range(NB):
                    if b not in covered:
                        z = work.tile([128, HP], f32, tag="red1")
                        nc.vector.memset(z[:], 0.0)
                        nc.sync.dma_start(s_high[128 * b : 128 * (b + 1), :], z[:])

                # -- low-half ELL: gather, reduce into s_low (SBUF) -------
                col = 0
                for bl in jobs0:
                    slots = sum(K0[b] for b in bl)
                    if slots == 0:
                        continue
                    nidx = 128 * slots
                    g = gjob.tile([128, slots, HP], bf16, tag="g0")
                    nc.gpsimd.dma_gather(
                        g[:],
                        tbl[:HALF, :],
                        idx_low[:, col : col + nidx // 16],
                        nidx,
                        nidx,
                        HP,
                        single_packet=False,
                    )
                    col += nidx // 16
                    off = 0
                    for b in bl:
                        k = K0[b]
                        if k == 0:
                            continue
                        nc.vector.reduce_sum(
                            out=s_low[:, HP * b : HP * (b + 1)],
                            in_=g[:, off : off + k, :].rearrange("p k d -> p d k"),
                            axis=mybir.AxisListType.X,
                        )
                        off += k
                covered = {b for bl in jobs0 for b in bl if K0[b] > 0}
                for b in range(NB):
                    if b not in covered:
                        nc.vector.memset(s_low[:, HP * b : HP * (b + 1)], 0.0)

                # -- realign high partials into table order ---------------
                nc.gpsimd.dma_gather(
                    s2[:].rearrange("p (b d) -> p b d", d=HP),
                    s_high[:],
                    idx_l2[:],
                    NT,
                    NT,
                    HP,
                    single_packet=False,
                )

                # -- post: out = relu(dinv * (S_low + S_high) + b) --------
                for b in range(NB):
                    comb = work.tile([128, HP], f32, tag="comb")
                    nc.vector.tensor_add(
                        comb[:],
                        s_low[:, HP * b : HP * (b + 1)],
                        s2[:, HP * b : HP * (b + 1)],
                    )
                    nc.vector.tensor_scalar_mul(comb[:], comb[:], dinv[:, b : b + 1])
                    pt = psum.tile([128, 128], f32, tag="ps")
                    nc.tensor.transpose(pt[:], comb[:], ident[:])
                    nc.scalar.activation(
                        out=hT[l][:, 128 * b : 128 * (b + 1)],
                        in_=pt[:HD, :],
                        func=mybir.ActivationFunctionType.Relu,
                        bias=bT[:, l : l + 1],
                    )

            # ---- JK projection: h = [h1 h2 h3] @ Wjk -> h_tbl (bf16) ----
            for b in range(NB):
                pj = psum.tile([128, HP], f32, tag="ps")
                for l in range(3):
                    nc.tensor.matmul(
                        pj[:],
                        lhsT=hT[l][:, 128 * b : 128 * (b + 1)],
                        rhs=wjk[l][:],
                        start=(l == 0),
                        stop=(l == 2),
                    )
                ht = work.tile([128, HP], bf16, tag="ht")
                lo = NLOC - 128 * b
                if 0 <= lo < 128:
                    nc.vector.memset(ht[:], 0.0)  # zero pad rows
                    nc.vector.tensor_copy(ht[:lo, :], pj[:lo, :])
                else:
                    nc.vector.tensor_copy(ht[:], pj[:])
                nc.sync.dma_start(h_tbl[128 * b : 128 * (b + 1), :], ht[:])

            # ---- pooling: per-core partial graph sums -------------------
            col = 0
            for b in range(GB):
                k = KP[b]
                nidx = 128 * k
                g = gjob.tile([128, k, HP], bf16, tag="gp")
                nc.gpsimd.dma_gather(
                    g[:], h_tbl[:], idx_pool[:, col : col + nidx // 16], nidx, nidx, HP,
                    single_packet=False,
                )
                col += nidx // 16
                emb = work.tile([128, HP], f32, tag="emb")
                nc.vector.reduce_sum(
                    out=emb[:],
                    in_=g[:].rearrange("p k d -> p d k"),
                    axis=mybir.AxisListType.X,
                )
                nc.sync.dma_start(ar_in[128 * b : 128 * (b + 1), :], emb[:])

            nc.gpsimd.collective_compute(
                "AllReduce",
                mybir.AluOpType.add,
                ins=[ar_in[:].opt()],
                outs=[ar_out[:].opt()],
                replica_groups=[list(range(NC))],
            )

            # ---- MLP head (redundant on every core) ---------------------
            for b in range(GB):
                emb = work.tile([128, HP], f32, tag="emb2")
                nc.sync.dma_start(emb[:], ar_out[128 * b : 128 * (b + 1), :])
                pb = work.tile([128, HP], f32, tag="pb")
                nc.sync.dma_start(pb[:], pbias_in[128 * b : 128 * (b + 1), :])
                nc.vector.tensor_add(emb[:], emb[:], pb[:])

                pe = psum.tile([128, 128], f32, tag="ps")
                nc.tensor.transpose(pe[:], emb[:], ident[:])
                embT = work.tile([128, 128], f32, tag="embT")
                nc.vector.tensor_copy(embT[:], pe[:])

                p1 = psum.tile([128, HP], f32, tag="ps")
                nc.tensor.matmul(
                    p1[:], lhsT=embT[:HD, :], rhs=wm1[:], start=True, stop=False
                )
                nc.tensor.matmul(
                    p1[:], lhsT=ones1[:1, :], rhs=bm1[:1, :], start=False, stop=True
                )
                r1 = work.tile([128, HP], f32, tag="r1")
                nc.scalar.activation(
                    out=r1[:], in_=p1[:], func=mybir.ActivationFunctionType.Relu
                )

                prt = psum.tile([128, 128], f32, tag="ps")
                nc.tensor.transpose(prt[:], r1[:], ident[:])
                r1T = work.tile([128, 128], f32, tag="r1T")
                nc.vector.tensor_copy(r1T[:], prt[:])

                p2 = psum.tile([128, HP], f32, tag="ps")
                nc.tensor.matmul(
                    p2[:], lhsT=r1T[:HD, :], rhs=wm2[:], start=True, stop=False
                )
                nc.tensor.matmul(
                    p2[:], lhsT=ones1[:1, :], rhs=bm2[:1, :], start=False, stop=True
                )
                ot = work.tile([128, HP], f32, tag="ot")
                nc.vector.tensor_copy(ot[:], p2[:])
                r0 = 128 * b
                rows = min(128, n_graphs - r0)
                if rows > 0:
                    nc.sync.dma_start(out_ext[r0 : r0 + rows, :], ot[:rows, :OUT])

    nc.compile()
    return nc


def _in_maps(p, x, W0, W1, W2, Wjk, bjk, Wm1, bm1, Wm2, bm2, b0, b1, b2):
    NT, NB, GB, NG, NLOC = p.NT, p.NB, p.GB, p.NG, p.NLOC
    HP = 128

    def padw(w, rows, dt):
        o = np.zeros((rows, HP), dt)
        w = np.asarray(w, np.float32)
        o[: w.shape[0], : w.shape[1]] = w
        return o

    w0 = padw(W0, IN_DIM, BF16)
    w1 = padw(W1, HID, BF16)
    w2 = padw(W2, HID, BF16)
    wjk = np.stack([padw(np.asarray(Wjk, np.float32)[96 * l : 96 * (l + 1)], HID, BF16) for l in range(3)])
    wm1 = padw(Wm1, HID, np.float32)
    wm2 = padw(Wm2, HID, np.float32)
    bT = np.stack([np.asarray(b, np.float32) for b in (b0, b1, b2)], axis=1)  # [HID,3]
    bm1p = np.zeros((1, HP), np.float32)
    bm1p[0, :HID] = np.asarray(bm1, np.float32)
    bm2p = np.zeros((1, HP), np.float32)
    bm2p[0, :OUT_DIM] = np.asarray(bm2, np.float32)
    pbias = np.zeros((NG, HP), np.float32)
    pbias[:, :HID] = p.cnt_total[:, None] * np.asarray(bjk, np.float32)[None, :]

    x = np.asarray(x, np.float32)
    maps = []
    for c in range(NC):
        xT = np.zeros((x.shape[1], NT), BF16)
        nodes = p.rank0_order[c] * NC + c
        xT[:, :NLOC] = x[nodes].T.astype(BF16)
        maps.append(
            dict(
                xT=xT,
                w0=w0, w1=w1, w2=w2, wjk=wjk, wm1=wm1, wm2=wm2,
                bT=bT, bm1=bm1p, bm2=bm2p, pbias=pbias,
                dinv=np.ascontiguousarray(p.dinv_tab[c]),
                idx_low=np.ascontiguousarray(p.idx_low[c]),
                idx_high=np.ascontiguousarray(p.idx_high[c]),
                idx_l2=np.ascontiguousarray(p.idx_l2[c]),
                idx_pool=np.ascontiguousarray(p.idx_pool[c]),
            )
        )
    return maps


def kernel_with_results(x, edge_index, batch, W0, b0, W1, b1, W2, b2, Wjk, bjk,
                        Wm1, bm1, Wm2, bm2, trace=False):
    p = _plan(edge_index, batch)
    nc = _build(_meta(p))
    maps = _in_maps(p, x, W0, W1, W2, Wjk, bjk, Wm1, bm1, Wm2, bm2, b0, b1, b2)
    res = run_bass_kernel_spmd(nc, maps, core_ids=list(range(NC)), trace=trace)
    return np.asarray(res.results[0]["out"], np.float32), res


def kernel(x, edge_index, batch, W0, b0, W1, b1, W2, b2, Wjk, bjk, Wm1, bm1,
           Wm2, bm2):
    out, _ = kernel_with_results(
        x, edge_index, batch, W0, b0, W1, b1, W2, b2, Wjk, bjk, Wm1, bm1, Wm2, bm2
    )
    return out
